# revision 44
# baseline (speedup 1.0000x reference)
"""Multi-head attention block (B=8, S=1024, D=1024, H=16) on 8 TRN2 NeuronCores.

Data-parallel over batch: core i computes batch element i end-to-end.
Per-core dataflow (bf16 compute, f32 PSUM accumulation; x/W pre-cast to
bf16 on the host):
  xT = host-transposed x
  qkT[nt,s] = W_qkv[:, :2048]^T @ x^T      (q rows pre-scaled by hd^-0.5)
  v[s,c]    = x @ W_qkv[:, 2048:]          (head-interleaved + ones col, M=65)
  per head-pair hp (heads A=2hp, B=2hp+1), per key-tile jt:
    scoresT[kj,qi]: A and B emitted adjacently as row-tiled K=64 matmuls at
      tile_position (0,0)/(64,0) -> the PE co-streams them (2x).
    expT = exp(scoresT)                    (ScalarE, PSUM -> SBUF bf16)
  outT[c,qi], Z[qi] = va^T @ expT          (ones column accumulates Z, M=65)
  outT /= Z                                (approx reciprocal + bcast)
  out = outT^T @ W_proj                    (v-bias + proj bias exact on host)
Emission is jt-granular round-robin (scores slot | attnv of prev pair |
qkv/v filler) so the in-order PE queue always has independent work while
ScalarE works through the exps; scores slots telescope across pair
boundaries.
"""

import sys

if "/opt/trn_rl_repo" not in sys.path:
    sys.path.insert(0, "/opt/trn_rl_repo")

import ml_dtypes
import numpy as np

P = 128
S = 1024
D = 1024
H = 16
HD = 64
N_CORES = 8
SCALE = HD ** -0.5
ST = S // P   # 8 s-tiles
DT = D // P   # 8 d-tiles (contraction tiles)

_CACHE = {}


def _build():
    if "nc" in _CACHE:
        return _CACHE["nc"]

    from contextlib import ExitStack

    import concourse.bass as bass  # noqa: F401
    import concourse.mybir as mybir
    import concourse.tile as tile
    from concourse.tile import add_dep_helper
    from concourse import bacc
    F32 = mybir.dt.float32
    BF = mybir.dt.bfloat16
    AluOp = mybir.AluOpType
    Act = mybir.ActivationFunctionType

    nc = bacc.Bacc(
        "TRN2", target_bir_lowering=False, debug=False, num_devices=N_CORES
    )

    x_d = nc.dram_tensor("x", [D, S], BF, kind="ExternalInput")  # x^T
    wqkv_d = nc.dram_tensor("W_qkv", [D, 3 * D], BF, kind="ExternalInput")
    bqkv_d = nc.dram_tensor("b_qkv", [3 * D], F32, kind="ExternalInput")
    wproj_d = nc.dram_tensor("W_proj", [D, D], BF, kind="ExternalInput")
    bproj_d = nc.dram_tensor("b_proj", [D], F32, kind="ExternalInput")
    out_d = nc.dram_tensor("out", [S, D], F32, kind="ExternalOutput")

    with tile.TileContext(nc) as tc, ExitStack() as ctx:
        const = ctx.enter_context(tc.tile_pool(name="const", bufs=1))
        persist = ctx.enter_context(tc.tile_pool(name="persist", bufs=1))
        # PSUM budget (8 banks): pq 2x[128,512] (2) + sp 2x[128,1024] (4)
        #                        + po 2x[65,512] (2)
        pq = ctx.enter_context(tc.tile_pool(name="pq", bufs=2, space="PSUM"))
        sp = ctx.enter_context(tc.tile_pool(name="sp", bufs=2, space="PSUM"))
        po = ctx.enter_context(tc.tile_pool(name="po", bufs=2, space="PSUM"))
        small = ctx.enter_context(tc.tile_pool(name="small", bufs=2))
        qk_pool = ctx.enter_context(tc.tile_pool(name="qk", bufs=4))
        exp_pool = ctx.enter_context(tc.tile_pool(name="exp", bufs=2))
        ob_pool = ctx.enter_context(tc.tile_pool(name="obp", bufs=2))

        # ---- constants ----
        zbias = const.tile([P, 1], F32)
        nc.gpsimd.memset(zbias[:], 0.0)
        bqcol = const.tile([P, 16], F32)  # b_qkv q,k permuted [p, nt] on host
        dumm = const.tile([1, 8], F32)
        nc.gpsimd.memset(dumm[:], 0.0)
        # preload the exp table set during the DMA prologue
        nc.scalar.activation(dumm[:], dumm[:], Act.Exp, bias=zbias[0:1])

        # ---- persistent tensors ----
        va = [persist.tile([P, H * (HD + 1)], BF, name=f"va{s8}") for s8 in range(ST)]
        outT = [persist.tile([P, S], BF, name=f"outT{t}") for t in range(DT)]
        xT = [persist.tile([P, S], BF, name=f"xT{t}") for t in range(DT)]
        WqkE = [persist.tile([P, 2 * P], BF, name=f"WqkE{t}") for t in range(DT)]
        Wqk = [persist.tile([P, 2 * D - 2 * P], BF, name=f"Wqk{t}", tag=f"wsh{t}")
               for t in range(DT)]
        Wv = [persist.tile([P, D], BF, name=f"Wv{t}") for t in range(DT)]
        Wp = [persist.tile([P, D], BF, name=f"Wp{t}", tag=f"wsh{t}") for t in range(DT)]

        for s8 in range(ST):
            # only the per-head ones columns (col 64 of each 65-block)
            nc.gpsimd.memset(
                va[s8][:].rearrange("p (h c) -> p h c", c=HD + 1)[:, :, HD : HD + 1],
                1.0,
            )

        # ---- DMA prologue (order = arrival priority) ----
        for dt2 in range(DT):
            nc.sync.dma_start(
                WqkE[dt2][:, 0:P], wqkv_d[dt2 * P : (dt2 + 1) * P, 0:P]
            )
            nc.sync.dma_start(
                WqkE[dt2][:, P : 2 * P], wqkv_d[dt2 * P : (dt2 + 1) * P, D : D + P]
            )
            nc.sync.dma_start(xT[dt2][:], x_d[dt2 * P : (dt2 + 1) * P, :])
        nc.sync.dma_start(bqcol[:], bqkv_d[: 2 * D].rearrange("(p t) -> p t", t=16))
        for dt2 in range(DT):
            nc.sync.dma_start(Wv[dt2][:], wqkv_d[dt2 * P : (dt2 + 1) * P, 2 * D :])
        for dt2 in range(DT):  # bulk q/k weights for pairs 1-7
            nc.sync.dma_start(
                Wqk[dt2][:, 0 : D - P], wqkv_d[dt2 * P : (dt2 + 1) * P, P : D]
            )
            nc.sync.dma_start(
                Wqk[dt2][:, D - P :], wqkv_d[dt2 * P : (dt2 + 1) * P, D + P : 2 * D]
            )
        def load_wp():
            for dt2 in range(DT):
                nc.sync.dma_start(Wp[dt2][:], wproj_d[dt2 * P : (dt2 + 1) * P, :])

        # ---- phase helpers (generators yield every ~4 matmuls) ----
        qkT = {}

        def w_ap(nt, dt2):
            """W_qkv column block for qk tile nt (0-7 q, 8-15 k)."""
            if nt == 0:
                return WqkE[dt2][:, 0:P]
            if nt == 8:
                return WqkE[dt2][:, P : 2 * P]
            col = (nt - 1) if nt < 8 else (DT - 1 + nt - 9)
            return Wqk[dt2][:, col * P : (col + 1) * P]

        def qkv_tile_chunks(hp):
            """qkT tiles for pair hp: q (scaled+bias) and k (bias)."""
            qt = qk_pool.tile([P, S], BF, name=f"q{hp}", tag="qk")
            kt = qk_pool.tile([P, S], BF, name=f"k{hp}", tag="qk")
            qkT[hp] = (qt, kt)
            for nt, dst in ((hp, qt), (8 + hp, kt)):
                for sh in range(2):
                    ps = pq.tile([P, 512], F32, name="ps_qk", tag="pq")
                    for dt2 in range(DT):
                        nc.tensor.matmul(
                            ps[:],
                            w_ap(nt, dt2),
                            xT[dt2][:, sh * 512 : (sh + 1) * 512],
                            start=(dt2 == 0),
                            stop=(dt2 == DT - 1),
                        )
                        if dt2 == 3:
                            yield
                    d = dst[:, sh * 512 : (sh + 1) * 512]
                    if nt < 8:  # q: (psum + b) * scale
                        nc.vector.tensor_scalar(
                            d, ps[:], bqcol[:, nt : nt + 1], SCALE,
                            AluOp.add, AluOp.mult,
                        )
                    else:  # k: psum + b
                        nc.vector.tensor_scalar_add(d, ps[:], bqcol[:, nt : nt + 1])
                    yield

        def v_chunks(s8_lo, s8_hi):
            """v = x @ Wv, head-interleaved into va (ones col at 64)."""
            for s8 in range(s8_lo, s8_hi):
                for sh in range(2):
                    ps = pq.tile([P, 512], F32, name="ps_v", tag="pq")
                    for dt2 in range(DT):
                        nc.tensor.matmul(
                            ps[:],
                            xT[dt2][:, s8 * P : (s8 + 1) * P],
                            Wv[dt2][:, sh * 512 : (sh + 1) * 512],
                            start=(dt2 == 0),
                            stop=(dt2 == DT - 1),
                        )
                        if dt2 == 3:
                            yield
                    nc.vector.tensor_copy(
                        va[s8][:, sh * 8 * (HD + 1) : (sh * 8 + 8) * (HD + 1)]
                        .rearrange("p (h c) -> p h c", c=HD + 1)[:, :, 0:HD],
                        ps[:].rearrange("p (h c) -> p h c", c=HD),
                    )
                    yield

        exp_tiles = {}

        def scores_jt(hp, jt):
            """scoresT + exp for pair hp, key-tile jt. A/B co-stream pairs.
            DVE stages PSUM->SBUF bf16 (fast bank recycle), ScalarE does one
            2048-wide exp from SBUF. expAB layout per jt: [A(1024) | B(1024)].
            """
            # exp layout per (qh, jt): [head A queries(512) | head B queries(512)]
            if jt == 0:
                e0 = exp_pool.tile([P, ST * S], BF, name=f"exp0_{hp}", tag="exp0")
                e1 = exp_pool.tile([P, ST * S], BF, name=f"exp1_{hp}", tag="exp1")
                exp_tiles[hp] = (e0, e1)
            exps = exp_tiles[hp]
            qt, kt = qkT[hp]
            for sh in range(2):
                # A and B of one query-half share ONE psum tile (2 banks) so
                # the scheduler cannot split the co-stream pair
                ps = sp.tile([P, S], F32, name="psS", tag="sp")
                nc.tensor.matmul(
                    ps[:, 0:512],
                    kt[0:64, jt * P : (jt + 1) * P],
                    qt[0:64, sh * 512 : (sh + 1) * 512],
                    tile_position=(0, 0),
                )
                nc.tensor.matmul(
                    ps[:, 512:1024],
                    kt[64:128, jt * P : (jt + 1) * P],
                    qt[64:128, sh * 512 : (sh + 1) * 512],
                    tile_position=(64, 0),
                )
                nc.scalar.activation(
                    exps[sh][:, jt * S : (jt + 1) * S], ps[:], Act.Exp,
                    bias=zbias[:],
                )

        def attnv_chunks(hp):
            """attn @ v for pair hp (M=65 with Z row). 8 yields of 4 mms."""
            exps = exp_tiles.pop(hp)
            for qh in range(2):
                for (hi, head) in ((0, 2 * hp), (1, 2 * hp + 1)):
                    ex = exps[qh]
                    pso = po.tile([HD + 1, 512], F32, name="pso", tag="po")
                    for jt in range(ST):
                        c0 = jt * S + hi * 512
                        nc.tensor.matmul(
                            pso[:],
                            va[jt][:, head * 65 : head * 65 + 65],
                            ex[:, c0 : c0 + 512],
                            start=(jt == 0),
                            stop=(jt == ST - 1),
                        )
                        if dt2_mid(jt):
                            yield
                    po_off = (head % 2) * 64
                    reg = outT[hp][po_off : po_off + 64, qh * 512 : (qh + 1) * 512]
                    nc.vector.tensor_copy(reg, pso[0:64, :])
                    zs = small.tile([1, 512], F32, name="zs", tag="zs")
                    nc.vector.tensor_copy(zs[:], pso[64:65, :])
                    rz = small.tile([1, 512], F32, name="rz", tag="rz")
                    nc.vector.reciprocal_approx_fast(out=rz[:], in_=zs[:])
                    bz = small.tile([P, 512], F32, name="bz", tag="bz")
                    nc.gpsimd.partition_broadcast(bz[:], rz[:])
                    nc.vector.tensor_mul(reg, reg, bz[po_off : po_off + 64, :])
                    yield

        def dt2_mid(j):
            return j == 3

        def proj_st(st):
            """output projection for s-tile st: 2 chains of 8 mms + drain."""
            for sh in range(2):
                ps = pq.tile([P, 512], F32, name="ps_p", tag="pq")
                for kt2 in range(DT):
                    nc.tensor.matmul(
                        ps[:],
                        outT[kt2][:, st * P : (st + 1) * P],
                        Wp[kt2][:, sh * 512 : (sh + 1) * 512],
                        start=(kt2 == 0),
                        stop=(kt2 == DT - 1),
                    )
                ob = ob_pool.tile([P, 512], F32, name="ob", tag="ob")
                nc.vector.tensor_copy(ob[:], ps[:])
                nc.sync.dma_start(
                    out_d[st * P : (st + 1) * P, sh * 512 : (sh + 1) * 512],
                    ob[:],
                )

        # ---- telescoped software-pipelined schedule ----
        def drain(gen, n):
            if gen is None:
                return 0
            k = 0
            for _ in range(n):
                try:
                    next(gen)
                    k += 1
                except StopIteration:
                    break
            return k

        # fill units of ~4 mms each: qkv pairs 1-7 + v, in needed-by order
        def mk_gen(kind, a):
            return qkv_tile_chunks(a) if kind == "qkv" else v_chunks(a, ST)

        fill_items = [["v", 2, None], ["qkv", 1, None]] + [
            ["qkv", h2, None] for h2 in range(2, ST)
        ]
        qkv_done = {0: True}

        def fill(n):
            got = 0
            while got < n and fill_items:
                item = fill_items[0]
                if item[2] is None:
                    item[2] = mk_gen(item[0], item[1])
                want = n - got
                t = drain(item[2], want)
                got += t
                if t < want:  # generator exhausted
                    if item[0] == "qkv":
                        qkv_done[item[1]] = True
                    fill_items.pop(0)
            return got

        def ensure_qkv(hp):
            while hp not in qkv_done and fill_items:
                fill(4)

        # prologue: qkv pair 0 (dense, DMA-gated), then first v slices
        for _ in qkv_tile_chunks(0):
            pass
        for _ in v_chunks(0, 2):
            pass

        att = None
        for hp in range(ST):
            ensure_qkv(hp)
            if hp == ST - 1:
                load_wp()
            for jt in range(ST):
                scores_jt(hp, jt)
                drain(att, 1)
                fill(4 if hp == 0 else 1)
            drain(att, 16)  # leftovers (normalize tails)
            att = attnv_chunks(hp)
        # epilogue: attnv(7) chases exp(7), then projection
        drain(att, 99)
        fill(999)
        for st in range(ST):
            proj_st(st)

    nc.compile()
    _CACHE["nc"] = nc
    return nc


def kernel(x, W_qkv, b_qkv, W_proj, b_proj, _trace=False):
    nc = _build()
    from concourse.bass_utils import run_bass_kernel_spmd

    bf = ml_dtypes.bfloat16
    wq = np.ascontiguousarray(W_qkv, dtype=np.float32).astype(bf)
    wp = np.ascontiguousarray(W_proj, dtype=np.float32).astype(bf)
    bq0 = np.asarray(b_qkv, dtype=np.float32)
    bq = np.concatenate(
        [np.ascontiguousarray(bq0[:2048].reshape(16, 128).T).ravel(), bq0[2048:]]
    ).astype(np.float32)
    bp = np.ascontiguousarray(b_proj, dtype=np.float32)
    in_maps = []
    for i in range(N_CORES):
        in_maps.append(
            {
                "x": np.ascontiguousarray(np.asarray(x[i], dtype=np.float32).T).astype(bf),
                "W_qkv": wq,
                "b_qkv": bq,
                "W_proj": wp,
                "b_proj": bp,
            }
        )
    res = run_bass_kernel_spmd(
        nc, in_maps, core_ids=list(range(N_CORES)), trace=_trace
    )
    out = np.stack([res.results[i]["out"] for i in range(N_CORES)], axis=0).astype(
        np.float32
    )
    # v-bias and proj-bias applied exactly on the host:
    # out = (attn + 1*bv) @ Wp + bp  ==  attn @ Wp  +  (bv @ Wp + bp)
    corr = np.asarray(b_qkv, np.float32)[2 * D :] @ np.asarray(W_proj, np.float32)
    corr = corr + np.asarray(b_proj, np.float32)
    if np.any(corr):
        out += corr[None, None, :]
    if _trace:
        _CACHE["last_results"] = res
    return out


# revision 46
# speedup vs baseline: 1.0072x; 1.0072x over previous
"""Multi-head attention block (B=8, S=1024, D=1024, H=16) on 8 TRN2 NeuronCores.

Data-parallel over batch: core i computes batch element i end-to-end.
Per-core dataflow (bf16 compute, f32 PSUM accumulation; x/W pre-cast to
bf16 on the host):
  xT = host-transposed x
  qkT[nt,s] = W_qkv[:, :2048]^T @ x^T      (q rows pre-scaled by hd^-0.5)
  v[s,c]    = x @ W_qkv[:, 2048:]          (head-interleaved + ones col, M=65)
  per head-pair hp (heads A=2hp, B=2hp+1), per key-tile jt:
    scoresT[kj,qi]: A and B emitted adjacently as row-tiled K=64 matmuls at
      tile_position (0,0)/(64,0) -> the PE co-streams them (2x).
    expT = exp(scoresT)                    (ScalarE, PSUM -> SBUF bf16)
  outT[c,qi], Z[qi] = va^T @ expT          (ones column accumulates Z, M=65)
  outT /= Z                                (approx reciprocal + bcast)
  out = outT^T @ W_proj                    (v-bias + proj bias exact on host)
Emission is jt-granular round-robin (scores slot | attnv of prev pair |
qkv/v filler) so the in-order PE queue always has independent work while
ScalarE works through the exps; scores slots telescope across pair
boundaries.
"""

import sys

if "/opt/trn_rl_repo" not in sys.path:
    sys.path.insert(0, "/opt/trn_rl_repo")

import ml_dtypes
import numpy as np

P = 128
S = 1024
D = 1024
H = 16
HD = 64
N_CORES = 8
SCALE = HD ** -0.5
ST = S // P   # 8 s-tiles
DT = D // P   # 8 d-tiles (contraction tiles)

_CACHE = {}


def _build():
    if "nc" in _CACHE:
        return _CACHE["nc"]

    from contextlib import ExitStack

    import concourse.bass as bass  # noqa: F401
    import concourse.mybir as mybir
    import concourse.tile as tile
    from concourse.tile import add_dep_helper
    from concourse import bacc
    F32 = mybir.dt.float32
    BF = mybir.dt.bfloat16
    AluOp = mybir.AluOpType
    Act = mybir.ActivationFunctionType

    nc = bacc.Bacc(
        "TRN2", target_bir_lowering=False, debug=False, num_devices=N_CORES
    )

    x_d = nc.dram_tensor("x", [D, S], BF, kind="ExternalInput")  # x^T
    wqkv_d = nc.dram_tensor("W_qkv", [D, 3 * D], BF, kind="ExternalInput")
    bqkv_d = nc.dram_tensor("b_qkv", [3 * D], F32, kind="ExternalInput")
    wproj_d = nc.dram_tensor("W_proj", [D, D], BF, kind="ExternalInput")
    bproj_d = nc.dram_tensor("b_proj", [D], F32, kind="ExternalInput")
    out_d = nc.dram_tensor("out", [S, D], F32, kind="ExternalOutput")

    with tile.TileContext(nc) as tc, ExitStack() as ctx:
        const = ctx.enter_context(tc.tile_pool(name="const", bufs=1))
        persist = ctx.enter_context(tc.tile_pool(name="persist", bufs=1))
        # PSUM budget (8 banks): pq 2x[128,512] (2) + sp 2x[128,1024] (4)
        #                        + po 2x[65,512] (2)
        pq = ctx.enter_context(tc.tile_pool(name="pq", bufs=2, space="PSUM"))
        sp = ctx.enter_context(tc.tile_pool(name="sp", bufs=2, space="PSUM"))
        po = ctx.enter_context(tc.tile_pool(name="po", bufs=2, space="PSUM"))
        small = ctx.enter_context(tc.tile_pool(name="small", bufs=2))
        qk_pool = ctx.enter_context(tc.tile_pool(name="qk", bufs=4))
        exp_pool = ctx.enter_context(tc.tile_pool(name="exp", bufs=2))
        ob_pool = ctx.enter_context(tc.tile_pool(name="obp", bufs=2))

        # ---- constants ----
        zbias = const.tile([P, 1], F32)
        nc.gpsimd.memset(zbias[:], 0.0)
        bqcol = const.tile([P, 16], F32)  # b_qkv q,k permuted [p, nt] on host
        dumm = const.tile([1, 8], F32)
        nc.gpsimd.memset(dumm[:], 0.0)
        # preload the exp table set during the DMA prologue
        nc.scalar.activation(dumm[:], dumm[:], Act.Exp, bias=zbias[0:1])

        # ---- persistent tensors ----
        va = [persist.tile([P, H * (HD + 1)], BF, name=f"va{s8}") for s8 in range(ST)]
        outT = [persist.tile([P, S], BF, name=f"outT{t}") for t in range(DT)]
        xT = [persist.tile([P, S], BF, name=f"xT{t}") for t in range(DT)]
        WqkE = [persist.tile([P, 2 * P], BF, name=f"WqkE{t}") for t in range(DT)]
        Wqk = [persist.tile([P, 2 * D - 2 * P], BF, name=f"Wqk{t}", tag=f"wsh{t}")
               for t in range(DT)]
        Wv = [persist.tile([P, D], BF, name=f"Wv{t}") for t in range(DT)]
        Wp = [persist.tile([P, D], BF, name=f"Wp{t}", tag=f"wsh{t}") for t in range(DT)]

        for s8 in range(ST):
            # only the per-head ones columns (col 64 of each 65-block)
            nc.gpsimd.memset(
                va[s8][:].rearrange("p (h c) -> p h c", c=HD + 1)[:, :, HD : HD + 1],
                1.0,
            )

        # ---- DMA prologue (order = arrival priority) ----
        for dt2 in range(DT):
            nc.sync.dma_start(
                WqkE[dt2][:, 0:P], wqkv_d[dt2 * P : (dt2 + 1) * P, 0:P]
            )
            nc.sync.dma_start(
                WqkE[dt2][:, P : 2 * P], wqkv_d[dt2 * P : (dt2 + 1) * P, D : D + P]
            )
            nc.sync.dma_start(xT[dt2][:], x_d[dt2 * P : (dt2 + 1) * P, :])
        nc.sync.dma_start(bqcol[:], bqkv_d[: 2 * D].rearrange("(p t) -> p t", t=16))
        for dt2 in range(DT):
            nc.sync.dma_start(Wv[dt2][:], wqkv_d[dt2 * P : (dt2 + 1) * P, 2 * D :])
        for dt2 in range(DT):  # bulk q/k weights for pairs 1-7
            nc.sync.dma_start(
                Wqk[dt2][:, 0 : D - P], wqkv_d[dt2 * P : (dt2 + 1) * P, P : D]
            )
            nc.sync.dma_start(
                Wqk[dt2][:, D - P :], wqkv_d[dt2 * P : (dt2 + 1) * P, D + P : 2 * D]
            )
        def load_wp():
            for dt2 in range(DT):
                nc.sync.dma_start(Wp[dt2][:], wproj_d[dt2 * P : (dt2 + 1) * P, :])

        # ---- phase helpers (generators yield every ~4 matmuls) ----
        qkT = {}

        def w_ap(nt, dt2):
            """W_qkv column block for qk tile nt (0-7 q, 8-15 k)."""
            if nt == 0:
                return WqkE[dt2][:, 0:P]
            if nt == 8:
                return WqkE[dt2][:, P : 2 * P]
            col = (nt - 1) if nt < 8 else (DT - 1 + nt - 9)
            return Wqk[dt2][:, col * P : (col + 1) * P]

        def qkv_tile_chunks(hp):
            """qkT tiles for pair hp: q (scaled+bias) and k (bias)."""
            qt = qk_pool.tile([P, S], BF, name=f"q{hp}", tag="qk")
            kt = qk_pool.tile([P, S], BF, name=f"k{hp}", tag="qk")
            qkT[hp] = (qt, kt)
            for nt, dst in ((hp, qt), (8 + hp, kt)):
                for sh in range(2):
                    ps = pq.tile([P, 512], F32, name="ps_qk", tag="pq")
                    for dt2 in range(DT):
                        nc.tensor.matmul(
                            ps[:],
                            w_ap(nt, dt2),
                            xT[dt2][:, sh * 512 : (sh + 1) * 512],
                            start=(dt2 == 0),
                            stop=(dt2 == DT - 1),
                        )
                        if dt2 == 3:
                            yield
                    d = dst[:, sh * 512 : (sh + 1) * 512]
                    if nt < 8:  # q: (psum + b) * scale
                        nc.vector.tensor_scalar(
                            d, ps[:], bqcol[:, nt : nt + 1], SCALE,
                            AluOp.add, AluOp.mult,
                        )
                    else:  # k: psum + b
                        nc.vector.tensor_scalar_add(d, ps[:], bqcol[:, nt : nt + 1])
                    yield

        def v_chunks(s8_lo, s8_hi):
            """v = x @ Wv, head-interleaved into va (ones col at 64)."""
            for s8 in range(s8_lo, s8_hi):
                for sh in range(2):
                    ps = pq.tile([P, 512], F32, name="ps_v", tag="pq")
                    for dt2 in range(DT):
                        nc.tensor.matmul(
                            ps[:],
                            xT[dt2][:, s8 * P : (s8 + 1) * P],
                            Wv[dt2][:, sh * 512 : (sh + 1) * 512],
                            start=(dt2 == 0),
                            stop=(dt2 == DT - 1),
                        )
                        if dt2 == 3:
                            yield
                    nc.vector.tensor_copy(
                        va[s8][:, sh * 8 * (HD + 1) : (sh * 8 + 8) * (HD + 1)]
                        .rearrange("p (h c) -> p h c", c=HD + 1)[:, :, 0:HD],
                        ps[:].rearrange("p (h c) -> p h c", c=HD),
                    )
                    yield

        exp_tiles = {}

        def scores_jt(hp, jt):
            """scoresT + exp for pair hp, key-tile jt. A/B co-stream pairs.
            DVE stages PSUM->SBUF bf16 (fast bank recycle), ScalarE does one
            2048-wide exp from SBUF. expAB layout per jt: [A(1024) | B(1024)].
            """
            # exp layout per (qh, jt): [head A queries(512) | head B queries(512)]
            if jt == 0:
                e0 = exp_pool.tile([P, ST * S], BF, name=f"exp0_{hp}", tag="exp0")
                e1 = exp_pool.tile([P, ST * S], BF, name=f"exp1_{hp}", tag="exp1")
                exp_tiles[hp] = (e0, e1)
            exps = exp_tiles[hp]
            qt, kt = qkT[hp]
            for sh in range(2):
                # A and B of one query-half share ONE psum tile (2 banks) so
                # the scheduler cannot split the co-stream pair
                ps = sp.tile([P, S], F32, name="psS", tag="sp")
                nc.tensor.matmul(
                    ps[:, 0:512],
                    kt[0:64, jt * P : (jt + 1) * P],
                    qt[0:64, sh * 512 : (sh + 1) * 512],
                    tile_position=(0, 0),
                )
                nc.tensor.matmul(
                    ps[:, 512:1024],
                    kt[64:128, jt * P : (jt + 1) * P],
                    qt[64:128, sh * 512 : (sh + 1) * 512],
                    tile_position=(64, 0),
                )
                nc.scalar.activation(
                    exps[sh][:, jt * S : (jt + 1) * S], ps[:], Act.Exp,
                    bias=zbias[:],
                )

        def attnv_chunks(hp):
            """attn @ v for pair hp (M=65 with Z row). 8 yields of 4 mms."""
            exps = exp_tiles.pop(hp)
            for qh in range(2):
                for (hi, head) in ((0, 2 * hp), (1, 2 * hp + 1)):
                    ex = exps[qh]
                    pso = po.tile([HD + 1, 512], F32, name="pso", tag="po")
                    for jt in range(ST):
                        c0 = jt * S + hi * 512
                        nc.tensor.matmul(
                            pso[:],
                            va[jt][:, head * 65 : head * 65 + 65],
                            ex[:, c0 : c0 + 512],
                            start=(jt == 0),
                            stop=(jt == ST - 1),
                        )
                        if dt2_mid(jt):
                            yield
                    po_off = (head % 2) * 64
                    reg = outT[hp][po_off : po_off + 64, qh * 512 : (qh + 1) * 512]
                    zs = small.tile([1, 512], F32, name="zs", tag="zs")
                    nc.vector.tensor_copy(zs[:], pso[64:65, :])
                    rz = small.tile([1, 512], F32, name="rz", tag="rz")
                    nc.vector.reciprocal_approx_fast(out=rz[:], in_=zs[:])
                    bz = small.tile([P, 512], F32, name="bz", tag="bz")
                    nc.gpsimd.partition_broadcast(bz[:], rz[:])
                    nc.vector.tensor_mul(reg, pso[0:64, :], bz[po_off : po_off + 64, :])
                    yield

        def dt2_mid(j):
            return j == 3

        def proj_st(st):
            """output projection for s-tile st: 2 chains of 8 mms + drain."""
            for sh in range(2):
                ps = pq.tile([P, 512], F32, name="ps_p", tag="pq")
                for kt2 in range(DT):
                    nc.tensor.matmul(
                        ps[:],
                        outT[kt2][:, st * P : (st + 1) * P],
                        Wp[kt2][:, sh * 512 : (sh + 1) * 512],
                        start=(kt2 == 0),
                        stop=(kt2 == DT - 1),
                    )
                ob = ob_pool.tile([P, 512], F32, name="ob", tag="ob")
                nc.vector.tensor_copy(ob[:], ps[:])
                nc.sync.dma_start(
                    out_d[st * P : (st + 1) * P, sh * 512 : (sh + 1) * 512],
                    ob[:],
                )

        # ---- telescoped software-pipelined schedule ----
        def drain(gen, n):
            if gen is None:
                return 0
            k = 0
            for _ in range(n):
                try:
                    next(gen)
                    k += 1
                except StopIteration:
                    break
            return k

        # fill units of ~4 mms each: qkv pairs 1-7 + v, in needed-by order
        def mk_gen(kind, a):
            return qkv_tile_chunks(a) if kind == "qkv" else v_chunks(a, ST)

        fill_items = [["v", 2, None], ["qkv", 1, None]] + [
            ["qkv", h2, None] for h2 in range(2, ST)
        ]
        qkv_done = {0: True}

        def fill(n):
            got = 0
            while got < n and fill_items:
                item = fill_items[0]
                if item[2] is None:
                    item[2] = mk_gen(item[0], item[1])
                want = n - got
                t = drain(item[2], want)
                got += t
                if t < want:  # generator exhausted
                    if item[0] == "qkv":
                        qkv_done[item[1]] = True
                    fill_items.pop(0)
            return got

        def ensure_qkv(hp):
            while hp not in qkv_done and fill_items:
                fill(4)

        # prologue: qkv pair 0 (dense, DMA-gated), then first v slices
        for _ in qkv_tile_chunks(0):
            pass
        for _ in v_chunks(0, 2):
            pass

        att = None
        for hp in range(ST):
            ensure_qkv(hp)
            if hp == ST - 1:
                load_wp()
            for jt in range(ST):
                scores_jt(hp, jt)
                drain(att, 1)
                fill(4 if hp == 0 else 1)
            drain(att, 16)  # leftovers (normalize tails)
            att = attnv_chunks(hp)
        # epilogue: attnv(7) chases exp(7), then projection
        drain(att, 99)
        fill(999)
        for st in range(ST):
            proj_st(st)

    nc.compile()
    _CACHE["nc"] = nc
    return nc


def kernel(x, W_qkv, b_qkv, W_proj, b_proj, _trace=False):
    nc = _build()
    from concourse.bass_utils import run_bass_kernel_spmd

    bf = ml_dtypes.bfloat16
    wq = np.ascontiguousarray(W_qkv, dtype=np.float32).astype(bf)
    wp = np.ascontiguousarray(W_proj, dtype=np.float32).astype(bf)
    bq0 = np.asarray(b_qkv, dtype=np.float32)
    bq = np.concatenate(
        [np.ascontiguousarray(bq0[:2048].reshape(16, 128).T).ravel(), bq0[2048:]]
    ).astype(np.float32)
    bp = np.ascontiguousarray(b_proj, dtype=np.float32)
    in_maps = []
    for i in range(N_CORES):
        in_maps.append(
            {
                "x": np.ascontiguousarray(np.asarray(x[i], dtype=np.float32).T).astype(bf),
                "W_qkv": wq,
                "b_qkv": bq,
                "W_proj": wp,
                "b_proj": bp,
            }
        )
    res = run_bass_kernel_spmd(
        nc, in_maps, core_ids=list(range(N_CORES)), trace=_trace
    )
    out = np.stack([res.results[i]["out"] for i in range(N_CORES)], axis=0).astype(
        np.float32
    )
    # v-bias and proj-bias applied exactly on the host:
    # out = (attn + 1*bv) @ Wp + bp  ==  attn @ Wp  +  (bv @ Wp + bp)
    corr = np.asarray(b_qkv, np.float32)[2 * D :] @ np.asarray(W_proj, np.float32)
    corr = corr + np.asarray(b_proj, np.float32)
    if np.any(corr):
        out += corr[None, None, :]
    if _trace:
        _CACHE["last_results"] = res
    return out


# revision 47
# speedup vs baseline: 1.0181x; 1.0108x over previous
"""Multi-head attention block (B=8, S=1024, D=1024, H=16) on 8 TRN2 NeuronCores.

Data-parallel over batch: core i computes batch element i end-to-end.
Per-core dataflow (bf16 compute, f32 PSUM accumulation; x/W pre-cast to
bf16 on the host):
  xT = host-transposed x
  qkT[nt,s] = W_qkv[:, :2048]^T @ x^T      (q rows pre-scaled by hd^-0.5)
  v[s,c]    = x @ W_qkv[:, 2048:]          (head-interleaved + ones col, M=65)
  per head-pair hp (heads A=2hp, B=2hp+1), per key-tile jt:
    scoresT[kj,qi]: A and B emitted adjacently as row-tiled K=64 matmuls at
      tile_position (0,0)/(64,0) -> the PE co-streams them (2x).
    expT = exp(scoresT)                    (ScalarE, PSUM -> SBUF bf16)
  outT[c,qi], Z[qi] = va^T @ expT          (ones column accumulates Z, M=65)
  outT /= Z                                (approx reciprocal + bcast)
  out = outT^T @ W_proj                    (v-bias + proj bias exact on host)
Emission is jt-granular round-robin (scores slot | attnv of prev pair |
qkv/v filler) so the in-order PE queue always has independent work while
ScalarE works through the exps; scores slots telescope across pair
boundaries.
"""

import sys

if "/opt/trn_rl_repo" not in sys.path:
    sys.path.insert(0, "/opt/trn_rl_repo")

import ml_dtypes
import numpy as np

P = 128
S = 1024
D = 1024
H = 16
HD = 64
N_CORES = 8
SCALE = HD ** -0.5
ST = S // P   # 8 s-tiles
DT = D // P   # 8 d-tiles (contraction tiles)

_CACHE = {}


def _build():
    if "nc" in _CACHE:
        return _CACHE["nc"]

    from contextlib import ExitStack

    import concourse.bass as bass  # noqa: F401
    import concourse.mybir as mybir
    import concourse.tile as tile
    from concourse.tile import add_dep_helper
    from concourse import bacc
    F32 = mybir.dt.float32
    BF = mybir.dt.bfloat16
    AluOp = mybir.AluOpType
    Act = mybir.ActivationFunctionType

    nc = bacc.Bacc(
        "TRN2", target_bir_lowering=False, debug=False, num_devices=N_CORES
    )

    x_d = nc.dram_tensor("x", [D, S], BF, kind="ExternalInput")  # x^T
    wqkv_d = nc.dram_tensor("W_qkv", [D, 3 * D], BF, kind="ExternalInput")
    bqkv_d = nc.dram_tensor("b_qkv", [3 * D], F32, kind="ExternalInput")
    wproj_d = nc.dram_tensor("W_proj", [D, D], BF, kind="ExternalInput")
    bproj_d = nc.dram_tensor("b_proj", [D], F32, kind="ExternalInput")
    out_d = nc.dram_tensor("out", [S, D], F32, kind="ExternalOutput")

    with tile.TileContext(nc) as tc, ExitStack() as ctx:
        const = ctx.enter_context(tc.tile_pool(name="const", bufs=1))
        persist = ctx.enter_context(tc.tile_pool(name="persist", bufs=1))
        # PSUM budget (8 banks): pq 2x[128,512] (2) + sp 2x[128,1024] (4)
        #                        + po 2x[65,512] (2)
        pq = ctx.enter_context(tc.tile_pool(name="pq", bufs=2, space="PSUM"))
        sp = ctx.enter_context(tc.tile_pool(name="sp", bufs=2, space="PSUM"))
        po = ctx.enter_context(tc.tile_pool(name="po", bufs=2, space="PSUM"))
        small = ctx.enter_context(tc.tile_pool(name="small", bufs=2))
        qk_pool = ctx.enter_context(tc.tile_pool(name="qk", bufs=4))
        exp_pool = ctx.enter_context(tc.tile_pool(name="exp", bufs=2))
        ob_pool = ctx.enter_context(tc.tile_pool(name="obp", bufs=2))

        # ---- constants ----
        zbias = const.tile([P, 1], F32)
        nc.gpsimd.memset(zbias[:], 0.0)
        bqcol = const.tile([P, 16], F32)  # b_qkv q,k permuted [p, nt] on host
        dumm = const.tile([1, 8], F32)
        nc.gpsimd.memset(dumm[:], 0.0)
        # preload the exp table set during the DMA prologue
        nc.scalar.activation(dumm[:], dumm[:], Act.Exp, bias=zbias[0:1])

        # ---- persistent tensors ----
        va = [persist.tile([P, H * (HD + 1)], BF, name=f"va{s8}") for s8 in range(ST)]
        outT = [persist.tile([P, S], BF, name=f"outT{t}") for t in range(DT)]
        xT = [persist.tile([P, S], BF, name=f"xT{t}") for t in range(DT)]
        WqkE = [persist.tile([P, 2 * P], BF, name=f"WqkE{t}") for t in range(DT)]
        Wqk = [persist.tile([P, 2 * D - 2 * P], BF, name=f"Wqk{t}", tag=f"wsh{t}")
               for t in range(DT)]
        Wv = [persist.tile([P, D], BF, name=f"Wv{t}") for t in range(DT)]
        Wp = [persist.tile([P, D], BF, name=f"Wp{t}", tag=f"wsh{t}") for t in range(DT)]

        for s8 in range(ST):
            # only the per-head ones columns (col 64 of each 65-block)
            nc.gpsimd.memset(
                va[s8][:].rearrange("p (h c) -> p h c", c=HD + 1)[:, :, HD : HD + 1],
                1.0,
            )

        # ---- DMA prologue (order = arrival priority) ----
        for dt2 in range(DT):
            nc.sync.dma_start(
                WqkE[dt2][:, 0:P], wqkv_d[dt2 * P : (dt2 + 1) * P, 0:P]
            )
            nc.sync.dma_start(
                WqkE[dt2][:, P : 2 * P], wqkv_d[dt2 * P : (dt2 + 1) * P, D : D + P]
            )
            nc.sync.dma_start(xT[dt2][:], x_d[dt2 * P : (dt2 + 1) * P, :])
        nc.sync.dma_start(bqcol[:], bqkv_d[: 2 * D].rearrange("(p t) -> p t", t=16))
        for dt2 in range(DT):
            nc.sync.dma_start(Wv[dt2][:], wqkv_d[dt2 * P : (dt2 + 1) * P, 2 * D :])
        for dt2 in range(DT):  # bulk q/k weights for pairs 1-7
            nc.sync.dma_start(
                Wqk[dt2][:, 0 : D - P], wqkv_d[dt2 * P : (dt2 + 1) * P, P : D]
            )
            nc.sync.dma_start(
                Wqk[dt2][:, D - P :], wqkv_d[dt2 * P : (dt2 + 1) * P, D + P : 2 * D]
            )
        def load_wp():
            for dt2 in range(DT):
                nc.sync.dma_start(Wp[dt2][:], wproj_d[dt2 * P : (dt2 + 1) * P, :])

        # ---- phase helpers (generators yield every ~4 matmuls) ----
        qkT = {}

        def w_ap(nt, dt2):
            """W_qkv column block for qk tile nt (0-7 q, 8-15 k)."""
            if nt == 0:
                return WqkE[dt2][:, 0:P]
            if nt == 8:
                return WqkE[dt2][:, P : 2 * P]
            col = (nt - 1) if nt < 8 else (DT - 1 + nt - 9)
            return Wqk[dt2][:, col * P : (col + 1) * P]

        def qkv_tile_chunks(hp):
            """qkT tiles for pair hp: q (scaled+bias) and k (bias)."""
            qt = qk_pool.tile([P, S], BF, name=f"q{hp}", tag="qk")
            kt = qk_pool.tile([P, S], BF, name=f"k{hp}", tag="qk")
            qkT[hp] = (qt, kt)
            for nt, dst in ((hp, qt), (8 + hp, kt)):
                for sh in range(2):
                    ps = pq.tile([P, 512], F32, name="ps_qk", tag="pq")
                    for dt2 in range(DT):
                        nc.tensor.matmul(
                            ps[:],
                            w_ap(nt, dt2),
                            xT[dt2][:, sh * 512 : (sh + 1) * 512],
                            start=(dt2 == 0),
                            stop=(dt2 == DT - 1),
                        )
                        if dt2 == 3:
                            yield
                    d = dst[:, sh * 512 : (sh + 1) * 512]
                    if nt < 8:  # q: (psum + b) * scale
                        nc.vector.tensor_scalar(
                            d, ps[:], bqcol[:, nt : nt + 1], SCALE,
                            AluOp.add, AluOp.mult,
                        )
                    else:  # k: psum + b
                        nc.vector.tensor_scalar_add(d, ps[:], bqcol[:, nt : nt + 1])
                    yield

        def v_chunks(s8_lo, s8_hi):
            """v = x @ Wv, head-interleaved into va (ones col at 64)."""
            for s8 in range(s8_lo, s8_hi):
                for sh in range(2):
                    ps = pq.tile([P, 512], F32, name="ps_v", tag="pq")
                    for dt2 in range(DT):
                        nc.tensor.matmul(
                            ps[:],
                            xT[dt2][:, s8 * P : (s8 + 1) * P],
                            Wv[dt2][:, sh * 512 : (sh + 1) * 512],
                            start=(dt2 == 0),
                            stop=(dt2 == DT - 1),
                        )
                        if dt2 == 3:
                            yield
                    nc.vector.tensor_copy(
                        va[s8][:, sh * 8 * (HD + 1) : (sh * 8 + 8) * (HD + 1)]
                        .rearrange("p (h c) -> p h c", c=HD + 1)[:, :, 0:HD],
                        ps[:].rearrange("p (h c) -> p h c", c=HD),
                    )
                    yield

        exp_tiles = {}

        def scores_jt(hp, jt):
            """scoresT + exp for pair hp, key-tile jt. A/B co-stream pairs.
            DVE stages PSUM->SBUF bf16 (fast bank recycle), ScalarE does one
            2048-wide exp from SBUF. expAB layout per jt: [A(1024) | B(1024)].
            """
            # exp layout per (qh, jt): [head A queries(512) | head B queries(512)]
            if jt == 0:
                e0 = exp_pool.tile([P, ST * S], BF, name=f"exp0_{hp}", tag="exp0")
                e1 = exp_pool.tile([P, ST * S], BF, name=f"exp1_{hp}", tag="exp1")
                exp_tiles[hp] = (e0, e1)
            exps = exp_tiles[hp]
            qt, kt = qkT[hp]
            for sh in range(2):
                # A and B of one query-half share ONE psum tile (2 banks) so
                # the scheduler cannot split the co-stream pair
                ps = sp.tile([P, S], F32, name="psS", tag="sp")
                nc.tensor.matmul(
                    ps[:, 0:512],
                    kt[0:64, jt * P : (jt + 1) * P],
                    qt[0:64, sh * 512 : (sh + 1) * 512],
                    tile_position=(0, 0),
                )
                nc.tensor.matmul(
                    ps[:, 512:1024],
                    kt[64:128, jt * P : (jt + 1) * P],
                    qt[64:128, sh * 512 : (sh + 1) * 512],
                    tile_position=(64, 0),
                )
                nc.scalar.activation(
                    exps[sh][:, jt * S : (jt + 1) * S], ps[:], Act.Exp,
                    bias=zbias[:],
                )

        def attnv_chunks(hp):
            """attn @ v for pair hp (M=65 with Z row). 8 yields of 4 mms."""
            exps = exp_tiles.pop(hp)
            for qh in range(2):
                for (hi, head) in ((0, 2 * hp), (1, 2 * hp + 1)):
                    ex = exps[qh]
                    pso = po.tile([HD + 1, 512], F32, name="pso", tag="po")
                    for jt in range(ST):
                        c0 = jt * S + hi * 512
                        nc.tensor.matmul(
                            pso[:],
                            va[jt][:, head * 65 : head * 65 + 65],
                            ex[:, c0 : c0 + 512],
                            start=(jt == 0),
                            stop=(jt == ST - 1),
                        )
                        if dt2_mid(jt):
                            yield
                    po_off = (head % 2) * 64
                    reg = outT[hp][po_off : po_off + 64, qh * 512 : (qh + 1) * 512]
                    zs = small.tile([1, 512], F32, name="zs", tag="zs")
                    nc.vector.tensor_copy(zs[:], pso[64:65, :])
                    rz = small.tile([1, 512], F32, name="rz", tag="rz")
                    nc.vector.reciprocal_approx_fast(out=rz[:], in_=zs[:])
                    bz = small.tile([P, 512], F32, name="bz", tag="bz")
                    nc.gpsimd.partition_broadcast(bz[:], rz[:])
                    nc.vector.tensor_mul(reg, pso[0:64, :], bz[po_off : po_off + 64, :])
                    yield

        def dt2_mid(j):
            return j == 3

        def proj_st(st):
            """output projection for s-tile st: 2 chains of 8 mms + drain."""
            for sh in range(2):
                ps = pq.tile([P, 512], F32, name="ps_p", tag="pq")
                for kt2 in range(DT):
                    nc.tensor.matmul(
                        ps[:],
                        outT[kt2][:, st * P : (st + 1) * P],
                        Wp[kt2][:, sh * 512 : (sh + 1) * 512],
                        start=(kt2 == 0),
                        stop=(kt2 == DT - 1),
                    )
                ob = ob_pool.tile([P, 512], F32, name="ob", tag="ob")
                nc.vector.tensor_copy(ob[:], ps[:])
                nc.sync.dma_start(
                    out_d[st * P : (st + 1) * P, sh * 512 : (sh + 1) * 512],
                    ob[:],
                )

        # ---- telescoped software-pipelined schedule ----
        def drain(gen, n):
            if gen is None:
                return 0
            k = 0
            for _ in range(n):
                try:
                    next(gen)
                    k += 1
                except StopIteration:
                    break
            return k

        # fill units of ~4 mms each: qkv pairs 1-7 + v, in needed-by order
        def mk_gen(kind, a):
            return qkv_tile_chunks(a) if kind == "qkv" else v_chunks(a, ST)

        fill_items = [["v", 2, None], ["qkv", 1, None]] + [
            ["qkv", h2, None] for h2 in range(2, ST)
        ]
        qkv_done = {0: True}

        def fill(n):
            got = 0
            while got < n and fill_items:
                item = fill_items[0]
                if item[2] is None:
                    item[2] = mk_gen(item[0], item[1])
                want = n - got
                t = drain(item[2], want)
                got += t
                if t < want:  # generator exhausted
                    if item[0] == "qkv":
                        qkv_done[item[1]] = True
                    fill_items.pop(0)
            return got

        def ensure_qkv(hp):
            while hp not in qkv_done and fill_items:
                fill(4)

        # prologue: qkv pair 0 (dense, DMA-gated), then first v slices
        for _ in qkv_tile_chunks(0):
            pass
        for _ in v_chunks(0, 2):
            pass

        att = None
        for hp in range(ST):
            ensure_qkv(hp)
            if hp == ST - 1:
                load_wp()
            for jt in range(ST):
                scores_jt(hp, jt)
                drain(att, 1)
                if hp == 0:
                    # front-load scores slots so ScalarE starts early; bulk
                    # v/qkv fill rides the second half of pair 0
                    fill(2 if jt < 4 else 6)
                else:
                    fill(1)
            drain(att, 16)  # leftovers (normalize tails)
            att = attnv_chunks(hp)
        # epilogue: attnv(7) chases exp(7), then projection
        drain(att, 99)
        fill(999)
        for st in range(ST):
            proj_st(st)

    nc.compile()
    _CACHE["nc"] = nc
    return nc


def kernel(x, W_qkv, b_qkv, W_proj, b_proj, _trace=False):
    nc = _build()
    from concourse.bass_utils import run_bass_kernel_spmd

    bf = ml_dtypes.bfloat16
    wq = np.ascontiguousarray(W_qkv, dtype=np.float32).astype(bf)
    wp = np.ascontiguousarray(W_proj, dtype=np.float32).astype(bf)
    bq0 = np.asarray(b_qkv, dtype=np.float32)
    bq = np.concatenate(
        [np.ascontiguousarray(bq0[:2048].reshape(16, 128).T).ravel(), bq0[2048:]]
    ).astype(np.float32)
    bp = np.ascontiguousarray(b_proj, dtype=np.float32)
    in_maps = []
    for i in range(N_CORES):
        in_maps.append(
            {
                "x": np.ascontiguousarray(np.asarray(x[i], dtype=np.float32).T).astype(bf),
                "W_qkv": wq,
                "b_qkv": bq,
                "W_proj": wp,
                "b_proj": bp,
            }
        )
    res = run_bass_kernel_spmd(
        nc, in_maps, core_ids=list(range(N_CORES)), trace=_trace
    )
    out = np.stack([res.results[i]["out"] for i in range(N_CORES)], axis=0).astype(
        np.float32
    )
    # v-bias and proj-bias applied exactly on the host:
    # out = (attn + 1*bv) @ Wp + bp  ==  attn @ Wp  +  (bv @ Wp + bp)
    corr = np.asarray(b_qkv, np.float32)[2 * D :] @ np.asarray(W_proj, np.float32)
    corr = corr + np.asarray(b_proj, np.float32)
    if np.any(corr):
        out += corr[None, None, :]
    if _trace:
        _CACHE["last_results"] = res
    return out
